# revision 59
# baseline (speedup 1.0000x reference)
"""GCNConv + PReLU + L2-normalize + global_mean_pool on 8 trn2 NeuronCores.

Strategy (per sharding hint): nodes are partitioned across the 8 cores
(load-balanced by in-degree so every 128-node tile has a bounded edge
count); edges are partitioned by destination node.  Each core computes
g = dinv * (x @ W) for its node shard (bf16 x, slab-batched DMA, host
pre-transposed so no PE transpose), the g-table is AllGathered (bf16)
to every core's HBM, then each core gathers source rows for its edges
(InstDMAGatherAnt) and scatter-adds them via one-hot matmuls into PSUM,
followed by the self-loop/bias/PReLU/L2-normalize epilogue and a
pooling matmul.  Per-graph partial sums are AllReduced and divided by
the per-graph node counts.

Math identity: with deg = in_degree + 1 and dinv = deg^-1/2,
  out[d] = dinv[d] * ( sum_{s->d} dinv[s]*h[s] + dinv[d]*h[d] ) + b
so the per-edge norm dinv[s]*dinv[d] never needs to be materialized.
The self term dinv[d]*h[d] comes from the resident bf16 g tile.

Gather layout ("grp" mode, the default): tiles are grouped B=4 at a
time; per (group, segment) ONE dma_gather call covers every tile's
edges, packed at SLOT granularity (each (tile,seg) block is padded only
to the cross-core max edge count, not to a multiple of 128).  Chunks
straddling a tile boundary are consumed by both tiles as separate
one-hot "instances" whose foreign slots read 255 and so contribute
zero.  ~100 calls/core of ~2.2k indices each over four <32768-row table
segments (int16 index limit), ~215k rows gathered per core.

KEY PERF FACTS (measured on HW this session):
- dma_gather desc-gen runs on ONE Q7 cpu pair selected by queue_num
  (ucode: cpu_id/2 == queue_num; even cpu = rx descs, odd = tx).  With
  num_swdge_queues=4 and calls spread over queues 0..3, desc-gen runs
  on all 4 pairs CONCURRENTLY: 7.67 -> 2.26 ns/idx (Q=1 -> Q=4).  The
  previous session's "queue rotation device fault" does NOT reproduce.
- The gather phase is desc-gen/issue bound, NOT HBM-bound: sequential
  vs random indices is only 1.87 vs 2.26 ns/idx at Q=4, so shrinking
  the payload (fp8 etc.) buys nothing; only fewer rows help.
- Calls up to 2560 indices work with dynamic_dma_scratch_size=65536
  (64KB SWDGE ring).  Ring size alone (without queue rotation) does
  NOT help - with the default 16KB ring the kernel ran at the same
  speed; the serialization was desc-gen, not ring space.
- Negative int16 indices (the documented "trailing negatives are
  dropped" path) HANG the device - do not use (GRP_NEGPAD=0).
- HWDGE sync.dma_start costs ~650ns issue per call: batch small DMAs
  into slabs (phase 1 uses SLAB=14 tiles per DMA).
- Queue assignment is greedy least-loaded by index count (~15us better
  than round-robin).

History: baseline (per-tile gathers, Q=1) 2337us -> grouped calls
804us -> +Q=4 rotation, slab phase1, slot-granular layout, bf16 x
-> 542us.  Phase1 ~20us, phase3 ~520us (~95% gather-bound at
2.4-2.5 ns/row in situ).  Tried and not kept: deeper oh/psum/epi
buffers (552us), gt bufs=2 (575us with old layout; bufs=3 is default).
"""

import numpy as np
import ml_dtypes

import concourse.bass as bass
import concourse.bacc as bacc
import concourse.tile as tile
import concourse.mybir as mybir
from concourse.masks import make_identity
from concourse.tile_rust import add_dep_helper
from concourse.bass_utils import run_bass_kernel_spmd

P = 128          # partitions / tile size
D = 128          # feature dim
G = 128          # number of graphs
NCORES = 8
MAXSEG = 25088   # gather-table segment rows (must stay < 32768 for int16)

import os as _os
GRP_B = int(_os.environ.get("GRP_B", "4"))        # tiles per gather group
GRP_QCAP = int(_os.environ.get("GRP_QCAP", "32"))  # max chunks per gather call
GRP_SCRATCH = int(_os.environ.get("GRP_SCRATCH", "65536"))  # SWDGE ring bytes
GRP_NQ = int(_os.environ.get("GRP_NQ", "4"))      # SWDGE queues (Q7 cpu pairs)
GRP_GTBUFS = int(_os.environ.get("GRP_GTBUFS", "3"))  # gather buffer depth
MODE = _os.environ.get("KMODE", "grp")

F32 = mybir.dt.float32
BF16 = mybir.dt.bfloat16
I16 = mybir.dt.int16
AF = mybir.ActivationFunctionType
OP = mybir.AluOpType


# ----------------------------------------------------------------------------
# Host-side packing: node permutation, edge partitioning, per-core arrays.
# ----------------------------------------------------------------------------
def pack_inputs(x, edge_index, batch, tiles_per_core=None):
    N = x.shape[0]
    E = edge_index.shape[1]
    src = edge_index[0].astype(np.int64)
    dst = edge_index[1].astype(np.int64)

    ntiles_min = -(-N // P)  # ceil
    ntiles = -(-ntiles_min // NCORES) * NCORES
    if tiles_per_core is not None:
        ntiles = tiles_per_core * NCORES
    TPC = ntiles // NCORES
    NPC = TPC * P
    NPAD = NCORES * NPC

    indeg = np.bincount(dst, minlength=N).astype(np.int64)
    deg = (indeg + 1).astype(np.float32)

    # ---- assign nodes to tiles: snake over tiles in descending in-degree ----
    order = np.argsort(-indeg, kind="stable")
    nrounds = NPAD // ntiles
    tile_seq = np.arange(ntiles)
    snake = np.empty((nrounds, ntiles), np.int64)
    snake[0::2] = tile_seq
    snake[1::2] = tile_seq[::-1]
    tile_of_slot = snake.reshape(-1)          # [NPAD]
    p_of_slot = np.repeat(np.arange(nrounds), ntiles)
    node_of_slot = np.full(NPAD, -1, np.int64)
    node_of_slot[:N] = order

    load = np.zeros(ntiles, np.int64)
    np.add.at(load, tile_of_slot[:N], indeg[order])

    # ---- assign tiles to cores: snake over cores in descending load ----
    tord = np.argsort(-load, kind="stable")
    core_of_tile = np.empty(ntiles, np.int64)
    tidx_of_tile = np.empty(ntiles, np.int64)
    cseq = np.arange(NCORES)
    for r in range(TPC):
        cs = cseq if r % 2 == 0 else cseq[::-1]
        tr = tord[r * NCORES:(r + 1) * NCORES]
        core_of_tile[tr] = cs
        tidx_of_tile[tr] = r

    # packed table row of each slot / node
    row_of_slot = (core_of_tile[tile_of_slot] * NPC
                   + tidx_of_tile[tile_of_slot] * P + p_of_slot)
    row_of_node = np.empty(N, np.int64)
    real = node_of_slot >= 0
    row_of_node[node_of_slot[real]] = row_of_slot[real]
    node_at_row = np.full(NCORES * NPC, -1, np.int64)
    node_at_row[row_of_slot] = node_of_slot

    # ---- edge slots ----
    K = max(1, int(-(-load.max() // P)))      # chunks per tile

    er = row_of_node[dst]
    ecore = er // NPC
    etile = (er % NPC) // P
    ep = er % P
    esrc = row_of_node[src]
    gtile = ecore * TPC + etile
    eo = np.lexsort((esrc, gtile))
    gt_s = gtile[eo]
    srow_s = esrc[eo]
    ep_s = ep[eo]
    tile_starts = np.searchsorted(gt_s, np.arange(ntiles))
    j = np.arange(E) - tile_starts[gt_s]
    assert j.max() < K * P
    core_s = gt_s // TPC
    t_s = gt_s % TPC

    srcidx = np.zeros((NCORES, P, TPC * K), np.int32)
    dstloc = np.full((NCORES, P, TPC * K), 255.0, ml_dtypes.bfloat16)
    srcidx[core_s, j % P, t_s * K + j // P] = srow_s.astype(np.int32)
    dstloc[core_s, j % P, t_s * K + j // P] = ep_s.astype(ml_dtypes.bfloat16)

    # ---- per-core node arrays ----
    nar = node_at_row.reshape(NCORES, NPC)
    xshT = np.zeros((NCORES, D, NPC), ml_dtypes.bfloat16)
    xsh = np.zeros((NCORES, NPC, D), np.float32)
    degsh = np.ones((NCORES, P, TPC), np.float32)
    batsh = np.zeros((NCORES, P, TPC), ml_dtypes.bfloat16)
    for c in range(NCORES):
        nc_ = nar[c]
        m = nc_ >= 0
        xsh[c][m] = x[nc_[m]]
        xshT[c] = xsh[c].T.astype(ml_dtypes.bfloat16)
        dg = np.ones(NPC, np.float32)
        dg[m] = deg[nc_[m]]
        degsh[c] = dg.reshape(TPC, P).T
        bt = np.full(NPC, 255.0, np.float32)
        bt[m] = batch[nc_[m]].astype(np.float32)
        batsh[c] = bt.reshape(TPC, P).T.astype(ml_dtypes.bfloat16)

    # ---- ant-mode (dma_gather) arrays: per-(tile, segment) chunk groups ----
    NSEG = max(1, -(-NPAD // MAXSEG))
    SEG = -(-NPAD // NSEG)
    assert SEG <= 32767
    eseg = esrc // SEG
    kg = (ecore * TPC + etile) * NSEG + eseg
    eo2 = np.lexsort((esrc, kg))
    kg_s = kg[eo2]
    srow2 = esrc[eo2]
    ep2 = ep[eo2]
    ngroups = NCORES * TPC * NSEG
    cnts = np.bincount(kg_s, minlength=ngroups).reshape(NCORES, TPC, NSEG)
    q = (-(-cnts.max(axis=0) // P)).astype(np.int64)      # [TPC, NSEG]
    offq = np.concatenate([np.zeros((TPC, 1), np.int64), np.cumsum(q, 1)], 1)
    K_eff = offq[:, -1]
    CBa = np.concatenate([[0], np.cumsum(K_eff)])
    CHKa = int(CBa[-1])
    gs = np.searchsorted(kg_s, np.arange(ngroups))
    i_in = np.arange(E) - gs[kg_s]
    c2 = kg_s // (TPC * NSEG)
    t2 = (kg_s // NSEG) % TPC
    s2 = kg_s % NSEG
    cb = CBa[t2] + offq[t2, s2]
    dstloca = np.full((NCORES, P, CHKa), 255.0, ml_dtypes.bfloat16)
    dstloca[c2, i_in % P, cb + i_in // P] = ep2.astype(ml_dtypes.bfloat16)
    idx16 = np.zeros((NCORES, 16, CHKa * 8), np.int16)
    idx16[c2, i_in % 16, cb * 8 + i_in // 16] = (
        srow2 - s2 * SEG).astype(np.int16)
    idx16 = np.tile(idx16, (1, 8, 1))

    cnt = np.bincount(batch.astype(np.int64), minlength=G).astype(np.float32)

    # ---- grp-mode (grouped dma_gather) arrays ------------------------------
    # Tiles are grouped B at a time; per (group, seg) ONE call gathers every
    # tile's edges packed at SLOT granularity (tile blocks padded only to the
    # cross-core max edge count, not to 128).  Chunks at tile boundaries are
    # shared: each (tile, chunk) overlap is a one-hot "instance" with foreign
    # slots masked to 255.  Self-loop g[d] comes from resident gres.
    B = GRP_B
    NG = -(-TPC // B)
    ecore_a = er // NPC
    etile_a = (er % NPC) // P
    ep_a = er % P
    eseg_a = esrc // SEG

    bucket = (ecore_a * TPC + etile_a) * NSEG + eseg_a
    bo = np.lexsort((esrc, bucket))
    b_s = bucket[bo]
    starts = np.searchsorted(b_s, np.arange(NCORES * TPC * NSEG))
    rank = np.arange(E) - starts[b_s]
    cnts3 = np.bincount(b_s, minlength=NCORES * TPC * NSEG
                        ).reshape(NCORES, TPC, NSEG)
    slots = cnts3.max(axis=0).astype(np.int64)        # [TPC, NSEG]

    lay = grp_layout2(slots, B)
    c3 = b_s // (TPC * NSEG)
    t3 = (b_s // NSEG) % TPC
    s3 = b_s % NSEG
    g3 = t3 // B
    sl = lay["soff"][t3, s3] + rank                   # slot within (gi,s)
    col = lay["cbase"][g3, s3] + sl // P              # global chunk col
    kloc = sl // P - lay["soff"][t3, s3] // P         # chunk within tile span
    inst = lay["ibase"][t3, s3] + kloc
    CHK2, NINST = lay["CHK"], lay["NINST"]
    dstg = np.full((NCORES, P, NINST), 255.0, ml_dtypes.bfloat16)
    dstg[c3, sl % P, inst] = ep_a[bo].astype(ml_dtypes.bfloat16)
    idx16g = np.zeros((NCORES, 16, CHK2 * 8), np.int16)
    idx16g[c3, sl % 16, col * 8 + (sl % P) // 16] = (
        esrc[bo] - s3 * SEG).astype(np.int16)
    # mark per-(core, gi, s) trailing pad slots negative so ucode drops them
    if _os.environ.get("GRP_NEGPAD", "0") == "1":
        lastsl = np.full((NCORES, NG, NSEG), -1, np.int64)
        np.maximum.at(lastsl, (c3, g3, s3), sl)
        for c in range(NCORES):
            for gi in range(NG):
                for s in range(NSEG):
                    ncg = int(lay["nck"][gi, s])
                    if ncg == 0:
                        continue
                    lo = int(lastsl[c, gi, s]) + 1
                    hi = ncg * P
                    if lo >= hi:
                        continue
                    i = np.arange(lo, hi)
                    cb = int(lay["cbase"][gi, s])
                    idx16g[c, i % 16, (cb + i // P) * 8 + (i % P) // 16] = -1
    idx16g = np.tile(idx16g, (1, 8, 1))

    return dict(K=K, TPC=TPC, NPC=NPC, NPAD=NPAD,
                srcidx=srcidx, dstloc=dstloc, xsh=xsh, xshT=xshT, degsh=degsh,
                batsh=batsh, cnt=cnt.reshape(G, 1),
                q=q, NSEG=NSEG, SEG=SEG, CHKa=CHKa,
                dstloca=dstloca, idx16=idx16,
                slots=slots, dstg=dstg, idx16g=idx16g)


def grp_layout2(slots, B):
    """Slot-granular chunk layout: order gi -> seg -> tile-in-group.

    slots[t,s] = cross-core max edge count of (tile,seg).  Returns dict with
    cbase[gi,s] global chunk col of each (group,seg) call block, nck[gi,s]
    chunks per call, soff[t,s] slot offset of tile within its block,
    ibase/icnt[t,s] one-hot instance columns, CHK total chunks, NINST total
    instances."""
    TPC, NSEG = slots.shape
    NG = -(-TPC // B)
    cbase = np.zeros((NG, NSEG), np.int64)
    nck = np.zeros((NG, NSEG), np.int64)
    soff = np.zeros((TPC, NSEG), np.int64)
    ibase = np.zeros((TPC, NSEG), np.int64)
    icnt = np.zeros((TPC, NSEG), np.int64)
    c = 0
    ii = 0
    for gi in range(NG):
        tlo, thi = gi * B, min((gi + 1) * B, TPC)
        for s in range(NSEG):
            off = 0
            for t in range(tlo, thi):
                soff[t, s] = off
                off += slots[t, s]
            nck[gi, s] = -(-off // P)
            for t in range(tlo, thi):
                if slots[t, s] > 0:
                    kl = soff[t, s] // P
                    kh = -(-(soff[t, s] + slots[t, s]) // P)
                else:
                    kl = kh = 0
                ibase[t, s] = ii
                icnt[t, s] = kh - kl
                ii += kh - kl
            cbase[gi, s] = c
            c += nck[gi, s]
    return dict(cbase=cbase, nck=nck, soff=soff, ibase=ibase, icnt=icnt,
                CHK=int(c), NINST=int(ii))


# ----------------------------------------------------------------------------
# Device program.
# ----------------------------------------------------------------------------
def build_program(K, TPC, NPAD, repeats=1, dbg=False, mode="indirect",
                  q=None, NSEG=1, SEG=None, QCAP=8, QROT=False,
                  single_packet=False, scratch=16384, phases="13"):
    NPC = TPC * P
    if mode == "ant":
        q = np.asarray(q)
        offq = np.concatenate([np.zeros((TPC, 1), np.int64),
                               np.cumsum(q, 1)], 1)
        K_eff = offq[:, -1]
        CBa = np.concatenate([[0], np.cumsum(K_eff)])
        CHKa = int(CBa[-1])
        KMAX = int(K_eff.max())
    elif mode == "grp":
        slots = np.asarray(q)                    # [TPC, NSEG] slot counts
        lay = grp_layout2(slots, GRP_B)
        cbase, nckL = lay["cbase"], lay["nck"]
        soffL, ibaseL, icntL = lay["soff"], lay["ibase"], lay["icnt"]
        CHKg, NINST = lay["CHK"], lay["NINST"]
        NG = -(-TPC // GRP_B)
        KMAXG = int(nckL.sum(axis=1).max())      # chunks in widest group
        KMAX = int(icntL.sum(axis=1).max())      # oh instances, widest tile
        QMAX = int(icntL.max())                  # iota width
    else:
        KMAX = K

    nc = bacc.Bacc("TRN2", target_bir_lowering=False, debug=False,
                   num_devices=NCORES,
                   dynamic_dma_scratch_size=scratch,
                   num_swdge_queues=(GRP_NQ if mode == "grp" else
                                     4 if (mode == "ant" and QROT) else 1))
    if dbg:
        dbg_gtab = nc.dram_tensor("dbg_gtab", [NPAD, D], BF16,
                                  kind="ExternalOutput")
        dbg_s = nc.dram_tensor("dbg_s", [P, TPC * D], F32,
                               kind="ExternalOutput")

    xsh = nc.dram_tensor("xsh", [D, NPC] if mode == "grp" else [NPC, D],
                         BF16 if mode == "grp" else F32,
                         kind="ExternalInput")
    degsh = nc.dram_tensor("degsh", [P, TPC], F32, kind="ExternalInput")
    batsh = nc.dram_tensor("batsh", [P, TPC], BF16, kind="ExternalInput")
    if mode == "ant":
        idx16 = nc.dram_tensor("idx16", [P, CHKa * 8], I16,
                               kind="ExternalInput")
        dstloc = nc.dram_tensor("dstloc", [P, CHKa], BF16,
                                kind="ExternalInput")
    elif mode == "grp":
        idx16 = nc.dram_tensor("idx16", [P, CHKg * 8], I16,
                               kind="ExternalInput")
        dstloc = nc.dram_tensor("dstloc", [P, NINST], BF16,
                                kind="ExternalInput")
    else:
        srcidx = nc.dram_tensor("srcidx", [P, TPC * K], mybir.dt.int32,
                                kind="ExternalInput")
        dstloc = nc.dram_tensor("dstloc", [P, TPC * K], BF16,
                                kind="ExternalInput")
    w_in = nc.dram_tensor("w", [D, D], F32, kind="ExternalInput")
    b_in = nc.dram_tensor("b", [1, D], F32, kind="ExternalInput")
    a_in = nc.dram_tensor("a", [1, D], F32, kind="ExternalInput")
    cnt_in = nc.dram_tensor("cnt", [G, 1], F32, kind="ExternalInput")
    pooled_out = nc.dram_tensor("pooled", [G, D], F32, kind="ExternalOutput")

    gshard = nc.dram_tensor("gshard", [NPC, D], BF16)
    gtable = nc.dram_tensor("gtable", [NPAD, D], BF16, addr_space="Shared")
    ar_in = nc.dram_tensor("ar_in", [G, D], F32)
    ar_out = nc.dram_tensor("ar_out", [G, D], F32, addr_space="Shared")

    with tile.TileContext(nc, num_cores=NCORES) as tc:
        with (
            tc.tile_pool(name="const", bufs=1) as constp,
            tc.tile_pool(name="resident", bufs=1) as resp,
            tc.tile_pool(name="meta", bufs=1) as metap,
        ):
            # ---- constants ----
            ident = constp.tile([P, P], F32)
            make_identity(nc, ident[:])
            w_t = constp.tile([D, D], F32)
            nc.sync.dma_start(out=w_t[:], in_=w_in[:])
            brow = constp.tile([1, D], F32)
            nc.sync.dma_start(out=brow[:], in_=b_in[:])
            bbc = constp.tile([P, D], F32)
            nc.gpsimd.partition_broadcast(bbc[:], brow[:])
            arow = constp.tile([1, D], F32)
            nc.sync.dma_start(out=arow[:], in_=a_in[:])
            abc = constp.tile([P, D], F32)
            nc.gpsimd.partition_broadcast(abc[:], arow[:])
            IW = QMAX if mode == "grp" else KMAX
            iota_dst = constp.tile([P, IW * P], BF16)
            nc.gpsimd.iota(iota_dst[:], pattern=[[0, IW], [1, P]], base=0,
                           channel_multiplier=0,
                           allow_small_or_imprecise_dtypes=True)
            iota_gr = constp.tile([P, P], BF16)
            nc.gpsimd.iota(iota_gr[:], pattern=[[1, P]], base=0,
                           channel_multiplier=0,
                           allow_small_or_imprecise_dtypes=True)
            cntc = constp.tile([G, 1], F32)
            nc.sync.dma_start(out=cntc[:], in_=cnt_in[:])
            rcnt = constp.tile([G, 1], F32)
            nc.vector.tensor_scalar_max(rcnt[:], cntc[:], 1.0)
            nc.vector.reciprocal(rcnt[:], rcnt[:])

            # ---- metadata / resident ----
            if mode == "ant":
                idx_t = metap.tile([P, CHKa * 8], I16)
                nc.sync.dma_start(out=idx_t[:], in_=idx16[:])
                dst_t = metap.tile([P, CHKa], BF16)
            elif mode == "grp":
                idx_t = metap.tile([P, CHKg * 8], I16)
                nc.sync.dma_start(out=idx_t[:], in_=idx16[:])
                dst_t = metap.tile([P, NINST], BF16)
            else:
                idx_t = metap.tile([P, TPC * K], mybir.dt.int32)
                nc.sync.dma_start(out=idx_t[:], in_=srcidx[:])
                dst_t = metap.tile([P, TPC * K], BF16)
            nc.sync.dma_start(out=dst_t[:], in_=dstloc[:])
            bat_t = metap.tile([P, TPC], BF16)
            nc.sync.dma_start(out=bat_t[:], in_=batsh[:])
            deg_t = metap.tile([P, TPC], F32)
            nc.sync.dma_start(out=deg_t[:], in_=degsh[:])
            dinv = resp.tile([P, TPC], F32)
            nc.scalar.sqrt(dinv[:], deg_t[:])
            nc.vector.reciprocal(dinv[:], dinv[:])
            gres = resp.tile([P, TPC * D], BF16 if mode == "grp" else F32)

            # ================= phase 1: g = dinv * (x @ W) =================
            def phase1():
                with (
                    tc.tile_pool(name="p1x", bufs=4) as p1x,
                    tc.tile_pool(name="p1ps", bufs=2, space="PSUM") as p1ps,
                    tc.tile_pool(name="p1hps",
                                 bufs=2 if mode == "grp" else 4,
                                 space="PSUM") as p1hps,
                ):
                    if mode == "grp":
                        # Slab-batched: one DMA in/out per SLAB tiles.  xsh is
                        # shipped transposed bf16 [D, NPC]; gshard written
                        # strided from contiguous gres slices.
                        SLAB = 14
                        wbf = p1x.tile([D, D], BF16, tag="wbf")
                        nc.vector.tensor_copy(wbf[:], w_t[:])
                        for t0 in range(0, TPC, SLAB):
                            ns = min(SLAB, TPC - t0)
                            xT = p1x.tile([P, SLAB * D], BF16, tag="xT")
                            nc.sync.dma_start(
                                out=xT[:, :ns * D],
                                in_=xsh[:, t0 * P:(t0 + ns) * P])
                            h_ps = p1hps.tile([P, SLAB * D], F32)
                            for j in range(ns):
                                nc.tensor.matmul(
                                    out=h_ps[:, j * D:(j + 1) * D],
                                    lhsT=xT[:, j * D:(j + 1) * D],
                                    rhs=wbf[:],
                                    start=True, stop=True,
                                    skip_group_check=True)
                            nc.vector.tensor_tensor(
                                out=gres[:, t0 * D:(t0 + ns) * D],
                                in0=h_ps[:, :ns * D],
                                in1=dinv[:, t0:t0 + ns]
                                    .to_broadcast([P, ns, D]),
                                op=OP.mult)
                            nc.sync.dma_start(
                                out=gshard[t0 * P:(t0 + ns) * P, :]
                                .rearrange("(t p) d -> p t d", p=P),
                                in_=gres[:, t0 * D:(t0 + ns) * D]
                                .rearrange("p (t d) -> p t d", t=ns))
                    else:
                        for t in range(TPC):
                            xt = p1x.tile([P, D], F32, tag="xt")
                            nc.sync.dma_start(out=xt[:],
                                              in_=xsh[t * P:(t + 1) * P, :])
                            xT_ps = p1ps.tile([P, D], F32)
                            nc.tensor.transpose(out=xT_ps[:], in_=xt[:],
                                                identity=ident[:])
                            xT = p1x.tile([P, D], F32, tag="xT")
                            nc.scalar.copy(xT[:], xT_ps[:])
                            h_ps = p1hps.tile([P, D], F32)
                            nc.tensor.matmul(out=h_ps[:], lhsT=xT[:],
                                             rhs=w_t[:],
                                             start=True, stop=True)
                            gsl = gres[:, t * D:(t + 1) * D]
                            nc.vector.tensor_scalar_mul(gsl, h_ps[:],
                                                        dinv[:, t:t + 1])
                            nc.gpsimd.dma_start(
                                out=gshard[t * P:(t + 1) * P, :], in_=gsl)

            if repeats > 1 and "1" in phases:
                with tc.For_i(0, repeats, 1):
                    phase1()
            else:
                phase1()

            # ---- AllGather the bf16 g table ----
            cc_ag = nc.gpsimd.collective_compute(
                "AllGather", OP.bypass,
                replica_groups=[list(range(NCORES))],
                ins=[gshard[:]], outs=[gtable[:]],
            )
            if dbg:
                nc.gpsimd.dma_start(out=dbg_gtab[:], in_=gtable[:])

            # ================= phase 3: SpMM + epilogue + pooling ===========
            with tc.tile_pool(name="poolacc", bufs=1, space="PSUM") as poolaccp:
                pooled_ps = poolaccp.tile([G, D], F32)

                def epilogue(t, s_ps, epip, smp, accum_pool, add_gres):
                    u = epip.tile([P, D], F32, tag="u")
                    if add_gres == "stream":
                        gst = epip.tile([P, D], BF16, tag="gst")
                        nc.sync.dma_start(out=gst[:],
                                          in_=gshard[t * P:(t + 1) * P, :])
                        nc.vector.tensor_tensor(
                            out=u[:], in0=s_ps[:], in1=gst[:], op=OP.add)
                        nc.scalar.mul(u[:], u[:], dinv[:, t:t + 1])
                    elif add_gres:
                        nc.vector.tensor_tensor(
                            out=u[:], in0=s_ps[:],
                            in1=gres[:, t * D:(t + 1) * D], op=OP.add)
                        nc.scalar.mul(u[:], u[:], dinv[:, t:t + 1])
                    else:
                        nc.scalar.mul(u[:], s_ps[:], dinv[:, t:t + 1])
                    nc.vector.tensor_tensor(out=u[:], in0=u[:],
                                            in1=bbc[:], op=OP.add)
                    pos = epip.tile([P, D], F32, tag="pos")
                    nc.scalar.activation(pos[:], u[:], AF.Relu)
                    neg = epip.tile([P, D], F32, tag="neg")
                    nc.vector.tensor_tensor(out=neg[:], in0=u[:],
                                            in1=pos[:], op=OP.subtract)
                    nc.vector.tensor_tensor(out=neg[:], in0=neg[:],
                                            in1=abc[:], op=OP.mult)
                    v = epip.tile([P, D], F32, tag="v")
                    nc.vector.tensor_tensor(out=v[:], in0=pos[:],
                                            in1=neg[:], op=OP.add)
                    sq = epip.tile([P, D], F32, tag="sq")
                    ss = smp.tile([P, 1], F32, tag="ss")
                    nc.scalar.activation(sq[:], v[:], AF.Square,
                                         accum_out=ss[:])
                    nc.scalar.sqrt(ss[:], ss[:])
                    nc.vector.tensor_scalar_max(ss[:], ss[:], 1e-12)
                    nc.vector.reciprocal(ss[:], ss[:])
                    o3 = epip.tile([P, D], F32, tag="o3")
                    nc.scalar.mul(o3[:], v[:], ss[:])
                    ohb = epip.tile([P, P], F32, tag="ohb")
                    nc.vector.tensor_tensor(
                        out=ohb[:],
                        in0=bat_t[:, t:t + 1].to_broadcast([P, P]),
                        in1=iota_gr[:], op=OP.is_equal)
                    nc.tensor.matmul(out=accum_pool[:], lhsT=ohb[:],
                                     rhs=o3[:], start=(t == 0),
                                     stop=(t == TPC - 1),
                                     skip_group_check=True)

                def phase3_grp(accum_pool):
                    with (
                        tc.tile_pool(name="gat", bufs=GRP_GTBUFS) as gatp,
                        tc.tile_pool(name="oh", bufs=2) as ohp,
                        tc.tile_pool(name="sps", bufs=2, space="PSUM") as spsp,
                        tc.tile_pool(name="epi", bufs=2) as epip,
                        tc.tile_pool(name="sm", bufs=2) as smp,
                    ):
                        qload = [0] * GRP_NQ
                        for gi in range(NG):
                            tlo = gi * GRP_B
                            thi = min((gi + 1) * GRP_B, TPC)
                            g0 = int(cbase[gi][0])    # first chunk of group
                            gt = gatp.tile([P, KMAXG * D], BF16, tag="gt")
                            for s in range(NSEG):
                                ncg = int(nckL[gi][s])
                                cb = int(cbase[gi][s])
                                o0 = cb - g0          # local chunk offset
                                a = 0
                                while a < ncg:
                                    bb = min(a + QCAP, ncg)
                                    qn = min(range(GRP_NQ),
                                             key=lambda i: qload[i])
                                    qload[qn] += bb - a
                                    gin = nc.gpsimd.dma_gather(
                                        gt[:, (o0 + a) * D:(o0 + bb) * D]
                                        .rearrange("p (k d) -> p k d",
                                                   k=bb - a),
                                        gtable[s * SEG:
                                               min((s + 1) * SEG, NPAD), :],
                                        idx_t[:, (cb + a) * 8:(cb + bb) * 8],
                                        (bb - a) * P, (bb - a) * P, D,
                                        elem_step=D,
                                        single_packet=single_packet,
                                        queue_num=qn)
                                    add_dep_helper(
                                        gin.ins, cc_ag.ins,
                                        reason="gather reads gtable")
                                    a = bb
                            if _os.environ.get("GRP_GATHER_ONLY") == "1":
                                if gi == NG - 1:  # keep pooled_ps written
                                    nc.tensor.matmul(out=accum_pool[:],
                                                     lhsT=ident[:],
                                                     rhs=w_t[:], start=True,
                                                     stop=True,
                                                     skip_group_check=True)
                                continue
                            for t in range(tlo, thi):
                                oh = ohp.tile([P, KMAX * P], BF16, tag="oh")
                                insts = []
                                pos = 0
                                for s in range(NSEG):
                                    ni = int(icntL[t][s])
                                    if ni == 0:
                                        continue
                                    ib = int(ibaseL[t][s])
                                    nc.vector.tensor_tensor(
                                        out=oh[:, pos * P:(pos + ni) * P],
                                        in0=dst_t[:, ib:ib + ni]
                                            .to_broadcast([P, ni, P]),
                                        in1=iota_dst[:, :ni * P],
                                        op=OP.is_equal)
                                    kl = int(soffL[t][s]) // P
                                    loc0 = int(cbase[gi][s]) - g0
                                    for j in range(ni):
                                        insts.append((pos + j,
                                                      loc0 + kl + j))
                                    pos += ni
                                s_ps = spsp.tile([P, D], F32, tag="s")
                                for i, (ko, kg_) in enumerate(insts):
                                    nc.tensor.matmul(
                                        out=s_ps[:],
                                        lhsT=oh[:, ko * P:(ko + 1) * P],
                                        rhs=gt[:, kg_ * D:(kg_ + 1) * D],
                                        start=(i == 0),
                                        stop=(i == len(insts) - 1))
                                epilogue(t, s_ps, epip, smp, accum_pool,
                                         add_gres=True)

                def phase3(accum_pool):
                    with (
                        tc.tile_pool(name="gat", bufs=3) as gatp,
                        tc.tile_pool(name="oh", bufs=2) as ohp,
                        tc.tile_pool(name="sps", bufs=2, space="PSUM") as spsp,
                        tc.tile_pool(name="epi", bufs=2) as epip,
                        tc.tile_pool(name="sm", bufs=2) as smp,
                    ):
                        for t in range(TPC):
                            KE = int(K_eff[t]) if mode == "ant" else K
                            DB = int(CBa[t]) if mode == "ant" else t * K
                            gt = gatp.tile([P, KMAX * D], BF16, tag="gt")
                            if mode == "ant":
                                for s in range(NSEG):
                                    qq = int(q[t][s])
                                    o0 = int(offq[t][s])
                                    cb8 = (int(CBa[t]) + o0) * 8
                                    a = 0
                                    while a < qq:
                                        bb = min(a + QCAP, qq)
                                        gi = nc.gpsimd.dma_gather(
                                            gt[:, (o0 + a) * D:(o0 + bb) * D]
                                            .rearrange("p (k d) -> p k d",
                                                       k=bb - a),
                                            gtable[s * SEG:
                                                   min((s + 1) * SEG, NPAD), :],
                                            idx_t[:, cb8 + a * 8:cb8 + bb * 8],
                                            (bb - a) * P, (bb - a) * P, D,
                                            elem_step=D,
                                            single_packet=single_packet,
                                            queue_num=(s % 4) if QROT else 0)
                                        add_dep_helper(
                                            gi.ins, cc_ag.ins,
                                            reason="gather reads gtable")
                                        a = bb
                            else:
                                for k in range(KE):
                                    gi = nc.gpsimd.indirect_dma_start(
                                        out=gt[:, k * D:(k + 1) * D],
                                        out_offset=None, in_=gtable[:],
                                        in_offset=bass.IndirectOffsetOnAxis(
                                            ap=idx_t[:, t * K + k:
                                                     t * K + k + 1],
                                            axis=0))
                                    add_dep_helper(gi.ins, cc_ag.ins,
                                                   reason="gather reads gtable")
                            oh = ohp.tile([P, KMAX * P], BF16, tag="oh")
                            nc.vector.tensor_tensor(
                                out=oh[:, :KE * P],
                                in0=dst_t[:, DB:DB + KE]
                                    .to_broadcast([P, KE, P]),
                                in1=iota_dst[:, :KE * P], op=OP.is_equal)
                            s_ps = spsp.tile([P, D], F32, tag="s")
                            for k in range(KE):
                                nc.tensor.matmul(
                                    out=s_ps[:],
                                    lhsT=oh[:, k * P:(k + 1) * P],
                                    rhs=gt[:, k * D:(k + 1) * D],
                                    start=(k == 0), stop=(k == KE - 1))
                            if dbg:
                                sdump = epip.tile([P, D], F32, tag="sdump")
                                nc.vector.tensor_copy(sdump[:], s_ps[:])
                                nc.sync.dma_start(
                                    out=dbg_s[:, t * D:(t + 1) * D],
                                    in_=sdump[:])
                            epilogue(t, s_ps, epip, smp, accum_pool,
                                     add_gres=True)

                p3 = phase3_grp if mode == "grp" else phase3
                if repeats > 1 and "3" in phases:
                    with tc.For_i(0, repeats, 1):
                        p3(pooled_ps)
                else:
                    p3(pooled_ps)

                with tc.tile_pool(name="fin", bufs=1) as finp:
                    pooled_sb = finp.tile([G, D], F32)
                    nc.vector.tensor_copy(pooled_sb[:], pooled_ps[:])
                    nc.sync.dma_start(out=ar_in[:], in_=pooled_sb[:])
                    nc.gpsimd.collective_compute(
                        "AllReduce", OP.add,
                        replica_groups=[list(range(NCORES))],
                        ins=[ar_in[:]], outs=[ar_out[:]],
                    )
                    red = finp.tile([G, D], F32)
                    nc.sync.dma_start(out=red[:], in_=ar_out[:])
                    fin = finp.tile([G, D], F32)
                    nc.scalar.mul(fin[:], red[:], rcnt[:])
                    nc.sync.dma_start(out=pooled_out[:], in_=fin[:])

    nc.compile()
    return nc


def make_in_maps(packed, W, b, prelu_a, mode="indirect"):
    W = np.ascontiguousarray(W, np.float32)
    b = np.ascontiguousarray(b, np.float32).reshape(1, D)
    a = np.ascontiguousarray(prelu_a, np.float32).reshape(1, D)
    maps = []
    for c in range(NCORES):
        m = {
            "xsh": (packed["xshT"] if mode == "grp" else packed["xsh"])[c],
            "degsh": packed["degsh"][c],
            "batsh": packed["batsh"][c],
            "w": W, "b": b, "a": a, "cnt": packed["cnt"],
        }
        if mode == "ant":
            m["idx16"] = packed["idx16"][c]
            m["dstloc"] = packed["dstloca"][c]
        elif mode == "grp":
            m["idx16"] = packed["idx16g"][c]
            m["dstloc"] = packed["dstg"][c]
        else:
            m["srcidx"] = packed["srcidx"][c]
            m["dstloc"] = packed["dstloc"][c]
        maps.append(m)
    return maps


def build_from_packed(packed, repeats=1):
    return build_program(packed["K"], packed["TPC"], packed["NPAD"],
                         repeats=repeats, mode=MODE,
                         q=packed["slots"] if MODE == "grp" else packed["q"],
                         NSEG=packed["NSEG"], SEG=packed["SEG"],
                         QCAP=GRP_QCAP if MODE == "grp" else 8,
                         scratch=GRP_SCRATCH if MODE == "grp" else 16384)


def kernel(x, edge_index, batch, W, b, prelu_a):
    x = np.asarray(x)
    edge_index = np.asarray(edge_index)
    batch = np.asarray(batch)
    packed = pack_inputs(x, edge_index, batch)
    nc = build_from_packed(packed)
    in_maps = make_in_maps(packed, np.asarray(W), np.asarray(b),
                           np.asarray(prelu_a), mode=MODE)
    res = run_bass_kernel_spmd(nc, in_maps, core_ids=list(range(NCORES)))
    return np.asarray(res.results[0]["pooled"], np.float32)



# revision 61
# speedup vs baseline: 1.0725x; 1.0725x over previous
"""GCNConv + PReLU + L2-normalize + global_mean_pool on 8 trn2 NeuronCores.

Strategy (per sharding hint): nodes are partitioned across the 8 cores
(load-balanced by in-degree so every 128-node tile has a bounded edge
count); edges are partitioned by destination node.  Each core computes
g = dinv * (x @ W) for its node shard (bf16 x, slab-batched DMA, host
pre-transposed so no PE transpose), the g-table is AllGathered (bf16)
to every core's HBM, then each core gathers source rows for its edges
(InstDMAGatherAnt) and scatter-adds them via one-hot matmuls into PSUM,
followed by the self-loop/bias/PReLU/L2-normalize epilogue and a
pooling matmul.  Per-graph partial sums are AllReduced and divided by
the per-graph node counts.

Math identity: with deg = in_degree + 1 and dinv = deg^-1/2,
  out[d] = dinv[d] * ( sum_{s->d} dinv[s]*h[s] + dinv[d]*h[d] ) + b
so the per-edge norm dinv[s]*dinv[d] never needs to be materialized.
The self term dinv[d]*h[d] comes from the resident bf16 g tile.

Gather layout ("grp" mode, the default): tiles are grouped B=4 at a
time; per (group, segment) ONE dma_gather call covers every tile's
edges, packed at SLOT granularity (each (tile,seg) block is padded only
to the cross-core max edge count, not to a multiple of 128).  Chunks
straddling a tile boundary are consumed by both tiles as separate
one-hot "instances" whose foreign slots read 255 and so contribute
zero.  ~100 calls/core of ~2.2k indices each over four <32768-row table
segments (int16 index limit), ~215k rows gathered per core.

KEY PERF FACTS (measured on HW this session):
- dma_gather desc-gen runs on ONE Q7 cpu pair selected by queue_num
  (ucode: cpu_id/2 == queue_num; even cpu = rx descs, odd = tx).  With
  num_swdge_queues=4 and calls spread over queues 0..3, desc-gen runs
  on all 4 pairs CONCURRENTLY: 7.67 -> 2.26 ns/idx (Q=1 -> Q=4).  The
  previous session's "queue rotation device fault" does NOT reproduce.
- The gather phase is desc-gen/issue bound, NOT HBM-bound: sequential
  vs random indices is only 1.87 vs 2.26 ns/idx at Q=4, so shrinking
  the payload (fp8 etc.) buys nothing; only fewer rows help.
- Calls up to 2560 indices work with dynamic_dma_scratch_size=65536
  (64KB SWDGE ring).  Ring size alone (without queue rotation) does
  NOT help - with the default 16KB ring the kernel ran at the same
  speed; the serialization was desc-gen, not ring space.
- Negative int16 indices (the documented "trailing negatives are
  dropped" path) HANG the device - do not use (GRP_NEGPAD=0).
- HWDGE sync.dma_start costs ~650ns issue per call: batch small DMAs
  into slabs (phase 1 uses SLAB=14 tiles per DMA).
- Queue assignment is greedy least-loaded by index count (~15us better
  than round-robin).

History: baseline (per-tile gathers, Q=1) 2337us -> grouped calls
804us -> +Q=4 rotation, slab phase1, slot-granular layout, bf16 x
-> 542us.  Phase1 ~20us, phase3 ~520us (~95% gather-bound at
2.4-2.5 ns/row in situ).  Tried and not kept: deeper oh/psum/epi
buffers (552us), gt bufs=2 (575us with old layout; bufs=3 is default).
"""

import numpy as np
import ml_dtypes

import concourse.bass as bass
import concourse.bacc as bacc
import concourse.tile as tile
import concourse.mybir as mybir
from concourse.masks import make_identity
from concourse.tile_rust import add_dep_helper
from concourse.bass_utils import run_bass_kernel_spmd

P = 128          # partitions / tile size
D = 128          # feature dim
G = 128          # number of graphs
NCORES = 8
MAXSEG = 25088   # gather-table segment rows (must stay < 32768 for int16)

import os as _os
GRP_B = int(_os.environ.get("GRP_B", "4"))        # tiles per gather group
GRP_QCAP = int(_os.environ.get("GRP_QCAP", "32"))  # max chunks per gather call
GRP_SCRATCH = int(_os.environ.get("GRP_SCRATCH", "65536"))  # SWDGE ring bytes
GRP_NQ = int(_os.environ.get("GRP_NQ", "4"))      # SWDGE queues (Q7 cpu pairs)
GRP_GTBUFS = int(_os.environ.get("GRP_GTBUFS", "3"))  # gather buffer depth
MODE = _os.environ.get("KMODE", "grp")

F32 = mybir.dt.float32
BF16 = mybir.dt.bfloat16
I16 = mybir.dt.int16
AF = mybir.ActivationFunctionType
OP = mybir.AluOpType


# ----------------------------------------------------------------------------
# Host-side packing: node permutation, edge partitioning, per-core arrays.
# ----------------------------------------------------------------------------
def pack_inputs(x, edge_index, batch, tiles_per_core=None):
    N = x.shape[0]
    E = edge_index.shape[1]
    src = edge_index[0].astype(np.int64)
    dst = edge_index[1].astype(np.int64)

    ntiles_min = -(-N // P)  # ceil
    ntiles = -(-ntiles_min // NCORES) * NCORES
    if tiles_per_core is not None:
        ntiles = tiles_per_core * NCORES
    TPC = ntiles // NCORES
    NPC = TPC * P
    NPAD = NCORES * NPC

    indeg = np.bincount(dst, minlength=N).astype(np.int64)
    deg = (indeg + 1).astype(np.float32)

    # ---- assign nodes to tiles: snake over tiles in descending in-degree ----
    order = np.argsort(-indeg, kind="stable")
    nrounds = NPAD // ntiles
    tile_seq = np.arange(ntiles)
    snake = np.empty((nrounds, ntiles), np.int64)
    snake[0::2] = tile_seq
    snake[1::2] = tile_seq[::-1]
    tile_of_slot = snake.reshape(-1)          # [NPAD]
    p_of_slot = np.repeat(np.arange(nrounds), ntiles)
    node_of_slot = np.full(NPAD, -1, np.int64)
    node_of_slot[:N] = order

    load = np.zeros(ntiles, np.int64)
    np.add.at(load, tile_of_slot[:N], indeg[order])

    # ---- assign tiles to cores: snake over cores in descending load ----
    tord = np.argsort(-load, kind="stable")
    core_of_tile = np.empty(ntiles, np.int64)
    tidx_of_tile = np.empty(ntiles, np.int64)
    cseq = np.arange(NCORES)
    for r in range(TPC):
        cs = cseq if r % 2 == 0 else cseq[::-1]
        tr = tord[r * NCORES:(r + 1) * NCORES]
        core_of_tile[tr] = cs
        tidx_of_tile[tr] = r

    # packed table row of each slot / node
    row_of_slot = (core_of_tile[tile_of_slot] * NPC
                   + tidx_of_tile[tile_of_slot] * P + p_of_slot)
    row_of_node = np.empty(N, np.int64)
    real = node_of_slot >= 0
    row_of_node[node_of_slot[real]] = row_of_slot[real]
    node_at_row = np.full(NCORES * NPC, -1, np.int64)
    node_at_row[row_of_slot] = node_of_slot

    # ---- edge slots ----
    K = max(1, int(-(-load.max() // P)))      # chunks per tile

    er = row_of_node[dst]
    ecore = er // NPC
    etile = (er % NPC) // P
    ep = er % P
    esrc = row_of_node[src]
    gtile = ecore * TPC + etile
    eo = np.lexsort((esrc, gtile))
    gt_s = gtile[eo]
    srow_s = esrc[eo]
    ep_s = ep[eo]
    tile_starts = np.searchsorted(gt_s, np.arange(ntiles))
    j = np.arange(E) - tile_starts[gt_s]
    assert j.max() < K * P
    core_s = gt_s // TPC
    t_s = gt_s % TPC

    srcidx = np.zeros((NCORES, P, TPC * K), np.int32)
    dstloc = np.full((NCORES, P, TPC * K), 255.0, ml_dtypes.bfloat16)
    srcidx[core_s, j % P, t_s * K + j // P] = srow_s.astype(np.int32)
    dstloc[core_s, j % P, t_s * K + j // P] = ep_s.astype(ml_dtypes.bfloat16)

    # ---- per-core node arrays ----
    nar = node_at_row.reshape(NCORES, NPC)
    xshT = np.zeros((NCORES, D, NPC), ml_dtypes.bfloat16)
    xsh = np.zeros((NCORES, NPC, D), np.float32)
    degsh = np.ones((NCORES, P, TPC), np.float32)
    batsh = np.zeros((NCORES, P, TPC), ml_dtypes.bfloat16)
    for c in range(NCORES):
        nc_ = nar[c]
        m = nc_ >= 0
        xsh[c][m] = x[nc_[m]]
        xshT[c] = xsh[c].T.astype(ml_dtypes.bfloat16)
        dg = np.ones(NPC, np.float32)
        dg[m] = deg[nc_[m]]
        degsh[c] = dg.reshape(TPC, P).T
        bt = np.full(NPC, 255.0, np.float32)
        bt[m] = batch[nc_[m]].astype(np.float32)
        batsh[c] = bt.reshape(TPC, P).T.astype(ml_dtypes.bfloat16)

    # ---- ant-mode (dma_gather) arrays: per-(tile, segment) chunk groups ----
    NSEG = max(1, -(-NPAD // MAXSEG))
    SEG = -(-NPAD // NSEG)
    assert SEG <= 32767
    eseg = esrc // SEG
    kg = (ecore * TPC + etile) * NSEG + eseg
    eo2 = np.lexsort((esrc, kg))
    kg_s = kg[eo2]
    srow2 = esrc[eo2]
    ep2 = ep[eo2]
    ngroups = NCORES * TPC * NSEG
    cnts = np.bincount(kg_s, minlength=ngroups).reshape(NCORES, TPC, NSEG)
    q = (-(-cnts.max(axis=0) // P)).astype(np.int64)      # [TPC, NSEG]
    offq = np.concatenate([np.zeros((TPC, 1), np.int64), np.cumsum(q, 1)], 1)
    K_eff = offq[:, -1]
    CBa = np.concatenate([[0], np.cumsum(K_eff)])
    CHKa = int(CBa[-1])
    gs = np.searchsorted(kg_s, np.arange(ngroups))
    i_in = np.arange(E) - gs[kg_s]
    c2 = kg_s // (TPC * NSEG)
    t2 = (kg_s // NSEG) % TPC
    s2 = kg_s % NSEG
    cb = CBa[t2] + offq[t2, s2]
    dstloca = np.full((NCORES, P, CHKa), 255.0, ml_dtypes.bfloat16)
    dstloca[c2, i_in % P, cb + i_in // P] = ep2.astype(ml_dtypes.bfloat16)
    idx16 = np.zeros((NCORES, 16, CHKa * 8), np.int16)
    idx16[c2, i_in % 16, cb * 8 + i_in // 16] = (
        srow2 - s2 * SEG).astype(np.int16)
    idx16 = np.tile(idx16, (1, 8, 1))

    cnt = np.bincount(batch.astype(np.int64), minlength=G).astype(np.float32)

    # ---- grp-mode (grouped dma_gather) arrays ------------------------------
    # Tiles are grouped B at a time; per (group, seg) ONE call gathers every
    # tile's edges packed at SLOT granularity (tile blocks padded only to the
    # cross-core max edge count, not to 128).  Chunks at tile boundaries are
    # shared: each (tile, chunk) overlap is a one-hot "instance" with foreign
    # slots masked to 255.  Self-loop g[d] comes from resident gres.
    B = GRP_B
    NG = -(-TPC // B)
    ecore_a = er // NPC
    etile_a = (er % NPC) // P
    ep_a = er % P
    eseg_a = esrc // SEG

    bucket = (ecore_a * TPC + etile_a) * NSEG + eseg_a
    bo = np.lexsort((esrc, bucket))
    b_s = bucket[bo]
    starts = np.searchsorted(b_s, np.arange(NCORES * TPC * NSEG))
    rank = np.arange(E) - starts[b_s]
    cnts3 = np.bincount(b_s, minlength=NCORES * TPC * NSEG
                        ).reshape(NCORES, TPC, NSEG)
    slots = cnts3.max(axis=0).astype(np.int64)        # [TPC, NSEG]

    lay = grp_layout2(slots, B)
    c3 = b_s // (TPC * NSEG)
    t3 = (b_s // NSEG) % TPC
    s3 = b_s % NSEG
    g3 = t3 // B
    sl = lay["soff"][t3, s3] + rank                   # slot within (gi,s)
    col = lay["cbase"][g3, s3] + sl // P              # global chunk col
    kloc = sl // P - lay["soff"][t3, s3] // P         # chunk within tile span
    inst = lay["ibase"][t3, s3] + kloc
    CHK2, NINST = lay["CHK"], lay["NINST"]
    dstg = np.full((NCORES, P, NINST), 255.0, ml_dtypes.bfloat16)
    dstg[c3, sl % P, inst] = ep_a[bo].astype(ml_dtypes.bfloat16)
    idx16g = np.zeros((NCORES, 16, CHK2 * 8), np.int16)
    idx16g[c3, sl % 16, col * 8 + (sl % P) // 16] = (
        esrc[bo] - s3 * SEG).astype(np.int16)
    # mark per-(core, gi, s) trailing pad slots negative so ucode drops them
    if _os.environ.get("GRP_NEGPAD", "0") == "1":
        lastsl = np.full((NCORES, NG, NSEG), -1, np.int64)
        np.maximum.at(lastsl, (c3, g3, s3), sl)
        for c in range(NCORES):
            for gi in range(NG):
                for s in range(NSEG):
                    ncg = int(lay["nck"][gi, s])
                    if ncg == 0:
                        continue
                    lo = int(lastsl[c, gi, s]) + 1
                    hi = ncg * P
                    if lo >= hi:
                        continue
                    i = np.arange(lo, hi)
                    cb = int(lay["cbase"][gi, s])
                    idx16g[c, i % 16, (cb + i // P) * 8 + (i % P) // 16] = -1
    idx16g = np.tile(idx16g, (1, 8, 1))

    return dict(K=K, TPC=TPC, NPC=NPC, NPAD=NPAD,
                srcidx=srcidx, dstloc=dstloc, xsh=xsh, xshT=xshT, degsh=degsh,
                batsh=batsh, cnt=cnt.reshape(G, 1),
                q=q, NSEG=NSEG, SEG=SEG, CHKa=CHKa,
                dstloca=dstloca, idx16=idx16,
                slots=slots, dstg=dstg, idx16g=idx16g)


def grp_layout2(slots, B):
    """Slot-granular chunk layout: order gi -> seg -> tile-in-group.

    slots[t,s] = cross-core max edge count of (tile,seg).  Returns dict with
    cbase[gi,s] global chunk col of each (group,seg) call block, nck[gi,s]
    chunks per call, soff[t,s] slot offset of tile within its block,
    ibase/icnt[t,s] one-hot instance columns, CHK total chunks, NINST total
    instances."""
    TPC, NSEG = slots.shape
    NG = -(-TPC // B)
    cbase = np.zeros((NG, NSEG), np.int64)
    nck = np.zeros((NG, NSEG), np.int64)
    soff = np.zeros((TPC, NSEG), np.int64)
    ibase = np.zeros((TPC, NSEG), np.int64)
    icnt = np.zeros((TPC, NSEG), np.int64)
    c = 0
    ii = 0
    for gi in range(NG):
        tlo, thi = gi * B, min((gi + 1) * B, TPC)
        for s in range(NSEG):
            off = 0
            for t in range(tlo, thi):
                soff[t, s] = off
                off += slots[t, s]
            nck[gi, s] = -(-off // P)
            for t in range(tlo, thi):
                if slots[t, s] > 0:
                    kl = soff[t, s] // P
                    kh = -(-(soff[t, s] + slots[t, s]) // P)
                else:
                    kl = kh = 0
                ibase[t, s] = ii
                icnt[t, s] = kh - kl
                ii += kh - kl
            cbase[gi, s] = c
            c += nck[gi, s]
    return dict(cbase=cbase, nck=nck, soff=soff, ibase=ibase, icnt=icnt,
                CHK=int(c), NINST=int(ii))


# ----------------------------------------------------------------------------
# Device program.
# ----------------------------------------------------------------------------
def build_program(K, TPC, NPAD, repeats=1, dbg=False, mode="indirect",
                  q=None, NSEG=1, SEG=None, QCAP=8, QROT=False,
                  single_packet=False, scratch=16384, phases="13"):
    NPC = TPC * P
    if mode == "ant":
        q = np.asarray(q)
        offq = np.concatenate([np.zeros((TPC, 1), np.int64),
                               np.cumsum(q, 1)], 1)
        K_eff = offq[:, -1]
        CBa = np.concatenate([[0], np.cumsum(K_eff)])
        CHKa = int(CBa[-1])
        KMAX = int(K_eff.max())
    elif mode == "grp":
        slots = np.asarray(q)                    # [TPC, NSEG] slot counts
        lay = grp_layout2(slots, GRP_B)
        cbase, nckL = lay["cbase"], lay["nck"]
        soffL, ibaseL, icntL = lay["soff"], lay["ibase"], lay["icnt"]
        CHKg, NINST = lay["CHK"], lay["NINST"]
        NG = -(-TPC // GRP_B)
        KMAXG = int(nckL.sum(axis=1).max())      # chunks in widest group
        KMAX = int(icntL.sum(axis=1).max())      # oh instances, widest tile
        QMAX = int(icntL.max())                  # iota width
    else:
        KMAX = K

    nc = bacc.Bacc("TRN2", target_bir_lowering=False, debug=False,
                   num_devices=NCORES,
                   dynamic_dma_scratch_size=scratch,
                   num_swdge_queues=(GRP_NQ if mode == "grp" else
                                     4 if (mode == "ant" and QROT) else 1))
    if dbg:
        dbg_gtab = nc.dram_tensor("dbg_gtab", [NPAD, D], BF16,
                                  kind="ExternalOutput")
        dbg_s = nc.dram_tensor("dbg_s", [P, TPC * D], F32,
                               kind="ExternalOutput")

    xsh = nc.dram_tensor("xsh", [D, NPC] if mode == "grp" else [NPC, D],
                         BF16 if mode == "grp" else F32,
                         kind="ExternalInput")
    degsh = nc.dram_tensor("degsh", [P, TPC], F32, kind="ExternalInput")
    batsh = nc.dram_tensor("batsh", [P, TPC], BF16, kind="ExternalInput")
    if mode == "ant":
        idx16 = nc.dram_tensor("idx16", [P, CHKa * 8], I16,
                               kind="ExternalInput")
        dstloc = nc.dram_tensor("dstloc", [P, CHKa], BF16,
                                kind="ExternalInput")
    elif mode == "grp":
        idx16 = nc.dram_tensor("idx16", [P, CHKg * 8], I16,
                               kind="ExternalInput")
        dstloc = nc.dram_tensor("dstloc", [P, NINST], BF16,
                                kind="ExternalInput")
    else:
        srcidx = nc.dram_tensor("srcidx", [P, TPC * K], mybir.dt.int32,
                                kind="ExternalInput")
        dstloc = nc.dram_tensor("dstloc", [P, TPC * K], BF16,
                                kind="ExternalInput")
    w_in = nc.dram_tensor("w", [D, D], F32, kind="ExternalInput")
    b_in = nc.dram_tensor("b", [1, D], F32, kind="ExternalInput")
    a_in = nc.dram_tensor("a", [1, D], F32, kind="ExternalInput")
    cnt_in = nc.dram_tensor("cnt", [G, 1], F32, kind="ExternalInput")
    pooled_out = nc.dram_tensor("pooled", [G, D], F32, kind="ExternalOutput")

    gshard = nc.dram_tensor("gshard", [NPC, D], BF16)
    gtable = nc.dram_tensor("gtable", [NPAD, D], BF16, addr_space="Shared")
    ar_in = nc.dram_tensor("ar_in", [G, D], F32)
    ar_out = nc.dram_tensor("ar_out", [G, D], F32, addr_space="Shared")

    with tile.TileContext(nc, num_cores=NCORES) as tc:
        with (
            tc.tile_pool(name="const", bufs=1) as constp,
            tc.tile_pool(name="resident", bufs=1) as resp,
            tc.tile_pool(name="meta", bufs=1) as metap,
        ):
            # ---- constants ----
            ident = constp.tile([P, P], F32)
            make_identity(nc, ident[:])
            w_t = constp.tile([D, D], F32)
            nc.sync.dma_start(out=w_t[:], in_=w_in[:])
            brow = constp.tile([1, D], F32)
            nc.sync.dma_start(out=brow[:], in_=b_in[:])
            bbc = constp.tile([P, D], F32)
            nc.gpsimd.partition_broadcast(bbc[:], brow[:])
            arow = constp.tile([1, D], F32)
            nc.sync.dma_start(out=arow[:], in_=a_in[:])
            abc = constp.tile([P, D], F32)
            nc.gpsimd.partition_broadcast(abc[:], arow[:])
            IW = QMAX if mode == "grp" else KMAX
            iota_dst = constp.tile([P, IW * P], BF16)
            nc.gpsimd.iota(iota_dst[:], pattern=[[0, IW], [1, P]], base=0,
                           channel_multiplier=0,
                           allow_small_or_imprecise_dtypes=True)
            iota_gr = constp.tile([P, P], BF16)
            nc.gpsimd.iota(iota_gr[:], pattern=[[1, P]], base=0,
                           channel_multiplier=0,
                           allow_small_or_imprecise_dtypes=True)
            cntc = constp.tile([G, 1], F32)
            nc.sync.dma_start(out=cntc[:], in_=cnt_in[:])
            rcnt = constp.tile([G, 1], F32)
            nc.vector.tensor_scalar_max(rcnt[:], cntc[:], 1.0)
            nc.vector.reciprocal(rcnt[:], rcnt[:])

            # ---- metadata / resident ----
            if mode == "ant":
                idx_t = metap.tile([P, CHKa * 8], I16)
                nc.sync.dma_start(out=idx_t[:], in_=idx16[:])
                dst_t = metap.tile([P, CHKa], BF16)
            elif mode == "grp":
                idx_t = metap.tile([P, CHKg * 8], I16)
                nc.sync.dma_start(out=idx_t[:], in_=idx16[:])
                dst_t = metap.tile([P, NINST], BF16)
            else:
                idx_t = metap.tile([P, TPC * K], mybir.dt.int32)
                nc.sync.dma_start(out=idx_t[:], in_=srcidx[:])
                dst_t = metap.tile([P, TPC * K], BF16)
            nc.sync.dma_start(out=dst_t[:], in_=dstloc[:])
            bat_t = metap.tile([P, TPC], BF16)
            nc.sync.dma_start(out=bat_t[:], in_=batsh[:])
            deg_t = metap.tile([P, TPC], F32)
            nc.sync.dma_start(out=deg_t[:], in_=degsh[:])
            dinv = resp.tile([P, TPC], F32)
            nc.scalar.sqrt(dinv[:], deg_t[:])
            nc.vector.reciprocal(dinv[:], dinv[:])
            gres = resp.tile([P, TPC * D], BF16 if mode == "grp" else F32)

            # ================= phase 1: g = dinv * (x @ W) =================
            def phase1():
                with (
                    tc.tile_pool(name="p1x", bufs=4) as p1x,
                    tc.tile_pool(name="p1ps", bufs=2, space="PSUM") as p1ps,
                    tc.tile_pool(name="p1hps", bufs=4, space="PSUM") as p1hps,
                ):
                    if mode == "grp":
                        # Slab-batched: one DMA in/out per SLAB tiles.  xsh is
                        # shipped transposed bf16 [D, NPC]; gshard written
                        # strided from contiguous gres slices.
                        SLAB = 14
                        wbf = p1x.tile([D, D], BF16, tag="wbf")
                        nc.vector.tensor_copy(wbf[:], w_t[:])
                        for t0 in range(0, TPC, SLAB):
                            ns = min(SLAB, TPC - t0)
                            xT = p1x.tile([P, SLAB * D], BF16, tag="xT")
                            nc.sync.dma_start(
                                out=xT[:, :ns * D],
                                in_=xsh[:, t0 * P:(t0 + ns) * P])
                            for j in range(ns):
                                t = t0 + j
                                h_ps = p1hps.tile([P, D], F32)
                                nc.tensor.matmul(
                                    out=h_ps[:],
                                    lhsT=xT[:, j * D:(j + 1) * D],
                                    rhs=wbf[:],
                                    start=True, stop=True)
                                nc.vector.tensor_scalar_mul(
                                    gres[:, t * D:(t + 1) * D], h_ps[:],
                                    dinv[:, t:t + 1])
                            nc.sync.dma_start(
                                out=gshard[t0 * P:(t0 + ns) * P, :]
                                .rearrange("(t p) d -> p t d", p=P),
                                in_=gres[:, t0 * D:(t0 + ns) * D]
                                .rearrange("p (t d) -> p t d", t=ns))
                    else:
                        for t in range(TPC):
                            xt = p1x.tile([P, D], F32, tag="xt")
                            nc.sync.dma_start(out=xt[:],
                                              in_=xsh[t * P:(t + 1) * P, :])
                            xT_ps = p1ps.tile([P, D], F32)
                            nc.tensor.transpose(out=xT_ps[:], in_=xt[:],
                                                identity=ident[:])
                            xT = p1x.tile([P, D], F32, tag="xT")
                            nc.scalar.copy(xT[:], xT_ps[:])
                            h_ps = p1hps.tile([P, D], F32)
                            nc.tensor.matmul(out=h_ps[:], lhsT=xT[:],
                                             rhs=w_t[:],
                                             start=True, stop=True)
                            gsl = gres[:, t * D:(t + 1) * D]
                            nc.vector.tensor_scalar_mul(gsl, h_ps[:],
                                                        dinv[:, t:t + 1])
                            nc.gpsimd.dma_start(
                                out=gshard[t * P:(t + 1) * P, :], in_=gsl)

            if repeats > 1 and "1" in phases:
                with tc.For_i(0, repeats, 1):
                    phase1()
            else:
                phase1()

            # ---- AllGather the bf16 g table ----
            cc_ag = nc.gpsimd.collective_compute(
                "AllGather", OP.bypass,
                replica_groups=[list(range(NCORES))],
                ins=[gshard[:]], outs=[gtable[:]],
            )
            if dbg:
                nc.gpsimd.dma_start(out=dbg_gtab[:], in_=gtable[:])

            # ================= phase 3: SpMM + epilogue + pooling ===========
            with tc.tile_pool(name="poolacc", bufs=1, space="PSUM") as poolaccp:
                pooled_ps = poolaccp.tile([G, D], F32)

                def epilogue(t, s_ps, epip, smp, accum_pool, add_gres):
                    u = epip.tile([P, D], F32, tag="u")
                    if add_gres == "stream":
                        gst = epip.tile([P, D], BF16, tag="gst")
                        nc.sync.dma_start(out=gst[:],
                                          in_=gshard[t * P:(t + 1) * P, :])
                        nc.vector.tensor_tensor(
                            out=u[:], in0=s_ps[:], in1=gst[:], op=OP.add)
                        nc.scalar.mul(u[:], u[:], dinv[:, t:t + 1])
                    elif add_gres:
                        nc.vector.tensor_tensor(
                            out=u[:], in0=s_ps[:],
                            in1=gres[:, t * D:(t + 1) * D], op=OP.add)
                        nc.scalar.mul(u[:], u[:], dinv[:, t:t + 1])
                    else:
                        nc.scalar.mul(u[:], s_ps[:], dinv[:, t:t + 1])
                    nc.vector.tensor_tensor(out=u[:], in0=u[:],
                                            in1=bbc[:], op=OP.add)
                    pos = epip.tile([P, D], F32, tag="pos")
                    nc.scalar.activation(pos[:], u[:], AF.Relu)
                    neg = epip.tile([P, D], F32, tag="neg")
                    nc.vector.tensor_tensor(out=neg[:], in0=u[:],
                                            in1=pos[:], op=OP.subtract)
                    nc.vector.tensor_tensor(out=neg[:], in0=neg[:],
                                            in1=abc[:], op=OP.mult)
                    v = epip.tile([P, D], F32, tag="v")
                    nc.vector.tensor_tensor(out=v[:], in0=pos[:],
                                            in1=neg[:], op=OP.add)
                    sq = epip.tile([P, D], F32, tag="sq")
                    ss = smp.tile([P, 1], F32, tag="ss")
                    nc.scalar.activation(sq[:], v[:], AF.Square,
                                         accum_out=ss[:])
                    nc.scalar.sqrt(ss[:], ss[:])
                    nc.vector.tensor_scalar_max(ss[:], ss[:], 1e-12)
                    nc.vector.reciprocal(ss[:], ss[:])
                    o3 = epip.tile([P, D], F32, tag="o3")
                    nc.scalar.mul(o3[:], v[:], ss[:])
                    ohb = epip.tile([P, P], F32, tag="ohb")
                    nc.vector.tensor_tensor(
                        out=ohb[:],
                        in0=bat_t[:, t:t + 1].to_broadcast([P, P]),
                        in1=iota_gr[:], op=OP.is_equal)
                    nc.tensor.matmul(out=accum_pool[:], lhsT=ohb[:],
                                     rhs=o3[:], start=(t == 0),
                                     stop=(t == TPC - 1),
                                     skip_group_check=True)

                def phase3_grp(accum_pool):
                    with (
                        tc.tile_pool(name="gat", bufs=GRP_GTBUFS) as gatp,
                        tc.tile_pool(name="oh", bufs=2) as ohp,
                        tc.tile_pool(name="sps", bufs=2, space="PSUM") as spsp,
                        tc.tile_pool(name="epi", bufs=2) as epip,
                        tc.tile_pool(name="sm", bufs=2) as smp,
                    ):
                        qload = [0] * GRP_NQ
                        for gi in range(NG):
                            tlo = gi * GRP_B
                            thi = min((gi + 1) * GRP_B, TPC)
                            g0 = int(cbase[gi][0])    # first chunk of group
                            gt = gatp.tile([P, KMAXG * D], BF16, tag="gt")
                            for s in range(NSEG):
                                ncg = int(nckL[gi][s])
                                cb = int(cbase[gi][s])
                                o0 = cb - g0          # local chunk offset
                                a = 0
                                while a < ncg:
                                    bb = min(a + QCAP, ncg)
                                    qn = min(range(GRP_NQ),
                                             key=lambda i: qload[i])
                                    qload[qn] += bb - a
                                    gin = nc.gpsimd.dma_gather(
                                        gt[:, (o0 + a) * D:(o0 + bb) * D]
                                        .rearrange("p (k d) -> p k d",
                                                   k=bb - a),
                                        gtable[s * SEG:
                                               min((s + 1) * SEG, NPAD), :],
                                        idx_t[:, (cb + a) * 8:(cb + bb) * 8],
                                        (bb - a) * P, (bb - a) * P, D,
                                        elem_step=D,
                                        single_packet=single_packet,
                                        queue_num=qn)
                                    add_dep_helper(
                                        gin.ins, cc_ag.ins,
                                        reason="gather reads gtable")
                                    a = bb
                            if _os.environ.get("GRP_GATHER_ONLY") == "1":
                                if gi == NG - 1:  # keep pooled_ps written
                                    nc.tensor.matmul(out=accum_pool[:],
                                                     lhsT=ident[:],
                                                     rhs=w_t[:], start=True,
                                                     stop=True,
                                                     skip_group_check=True)
                                continue
                            for t in range(tlo, thi):
                                oh = ohp.tile([P, KMAX * P], BF16, tag="oh")
                                insts = []
                                pos = 0
                                for s in range(NSEG):
                                    ni = int(icntL[t][s])
                                    if ni == 0:
                                        continue
                                    ib = int(ibaseL[t][s])
                                    nc.vector.tensor_tensor(
                                        out=oh[:, pos * P:(pos + ni) * P],
                                        in0=dst_t[:, ib:ib + ni]
                                            .to_broadcast([P, ni, P]),
                                        in1=iota_dst[:, :ni * P],
                                        op=OP.is_equal)
                                    kl = int(soffL[t][s]) // P
                                    loc0 = int(cbase[gi][s]) - g0
                                    for j in range(ni):
                                        insts.append((pos + j,
                                                      loc0 + kl + j))
                                    pos += ni
                                s_ps = spsp.tile([P, D], F32, tag="s")
                                for i, (ko, kg_) in enumerate(insts):
                                    nc.tensor.matmul(
                                        out=s_ps[:],
                                        lhsT=oh[:, ko * P:(ko + 1) * P],
                                        rhs=gt[:, kg_ * D:(kg_ + 1) * D],
                                        start=(i == 0),
                                        stop=(i == len(insts) - 1))
                                epilogue(t, s_ps, epip, smp, accum_pool,
                                         add_gres=True)

                def phase3(accum_pool):
                    with (
                        tc.tile_pool(name="gat", bufs=3) as gatp,
                        tc.tile_pool(name="oh", bufs=2) as ohp,
                        tc.tile_pool(name="sps", bufs=2, space="PSUM") as spsp,
                        tc.tile_pool(name="epi", bufs=2) as epip,
                        tc.tile_pool(name="sm", bufs=2) as smp,
                    ):
                        for t in range(TPC):
                            KE = int(K_eff[t]) if mode == "ant" else K
                            DB = int(CBa[t]) if mode == "ant" else t * K
                            gt = gatp.tile([P, KMAX * D], BF16, tag="gt")
                            if mode == "ant":
                                for s in range(NSEG):
                                    qq = int(q[t][s])
                                    o0 = int(offq[t][s])
                                    cb8 = (int(CBa[t]) + o0) * 8
                                    a = 0
                                    while a < qq:
                                        bb = min(a + QCAP, qq)
                                        gi = nc.gpsimd.dma_gather(
                                            gt[:, (o0 + a) * D:(o0 + bb) * D]
                                            .rearrange("p (k d) -> p k d",
                                                       k=bb - a),
                                            gtable[s * SEG:
                                                   min((s + 1) * SEG, NPAD), :],
                                            idx_t[:, cb8 + a * 8:cb8 + bb * 8],
                                            (bb - a) * P, (bb - a) * P, D,
                                            elem_step=D,
                                            single_packet=single_packet,
                                            queue_num=(s % 4) if QROT else 0)
                                        add_dep_helper(
                                            gi.ins, cc_ag.ins,
                                            reason="gather reads gtable")
                                        a = bb
                            else:
                                for k in range(KE):
                                    gi = nc.gpsimd.indirect_dma_start(
                                        out=gt[:, k * D:(k + 1) * D],
                                        out_offset=None, in_=gtable[:],
                                        in_offset=bass.IndirectOffsetOnAxis(
                                            ap=idx_t[:, t * K + k:
                                                     t * K + k + 1],
                                            axis=0))
                                    add_dep_helper(gi.ins, cc_ag.ins,
                                                   reason="gather reads gtable")
                            oh = ohp.tile([P, KMAX * P], BF16, tag="oh")
                            nc.vector.tensor_tensor(
                                out=oh[:, :KE * P],
                                in0=dst_t[:, DB:DB + KE]
                                    .to_broadcast([P, KE, P]),
                                in1=iota_dst[:, :KE * P], op=OP.is_equal)
                            s_ps = spsp.tile([P, D], F32, tag="s")
                            for k in range(KE):
                                nc.tensor.matmul(
                                    out=s_ps[:],
                                    lhsT=oh[:, k * P:(k + 1) * P],
                                    rhs=gt[:, k * D:(k + 1) * D],
                                    start=(k == 0), stop=(k == KE - 1))
                            if dbg:
                                sdump = epip.tile([P, D], F32, tag="sdump")
                                nc.vector.tensor_copy(sdump[:], s_ps[:])
                                nc.sync.dma_start(
                                    out=dbg_s[:, t * D:(t + 1) * D],
                                    in_=sdump[:])
                            epilogue(t, s_ps, epip, smp, accum_pool,
                                     add_gres=True)

                p3 = phase3_grp if mode == "grp" else phase3
                if repeats > 1 and "3" in phases:
                    with tc.For_i(0, repeats, 1):
                        p3(pooled_ps)
                else:
                    p3(pooled_ps)

                with tc.tile_pool(name="fin", bufs=1) as finp:
                    pooled_sb = finp.tile([G, D], F32)
                    nc.vector.tensor_copy(pooled_sb[:], pooled_ps[:])
                    nc.sync.dma_start(out=ar_in[:], in_=pooled_sb[:])
                    nc.gpsimd.collective_compute(
                        "AllReduce", OP.add,
                        replica_groups=[list(range(NCORES))],
                        ins=[ar_in[:]], outs=[ar_out[:]],
                    )
                    red = finp.tile([G, D], F32)
                    nc.sync.dma_start(out=red[:], in_=ar_out[:])
                    fin = finp.tile([G, D], F32)
                    nc.scalar.mul(fin[:], red[:], rcnt[:])
                    nc.sync.dma_start(out=pooled_out[:], in_=fin[:])

    nc.compile()
    return nc


def make_in_maps(packed, W, b, prelu_a, mode="indirect"):
    W = np.ascontiguousarray(W, np.float32)
    b = np.ascontiguousarray(b, np.float32).reshape(1, D)
    a = np.ascontiguousarray(prelu_a, np.float32).reshape(1, D)
    maps = []
    for c in range(NCORES):
        m = {
            "xsh": (packed["xshT"] if mode == "grp" else packed["xsh"])[c],
            "degsh": packed["degsh"][c],
            "batsh": packed["batsh"][c],
            "w": W, "b": b, "a": a, "cnt": packed["cnt"],
        }
        if mode == "ant":
            m["idx16"] = packed["idx16"][c]
            m["dstloc"] = packed["dstloca"][c]
        elif mode == "grp":
            m["idx16"] = packed["idx16g"][c]
            m["dstloc"] = packed["dstg"][c]
        else:
            m["srcidx"] = packed["srcidx"][c]
            m["dstloc"] = packed["dstloc"][c]
        maps.append(m)
    return maps


def build_from_packed(packed, repeats=1):
    return build_program(packed["K"], packed["TPC"], packed["NPAD"],
                         repeats=repeats, mode=MODE,
                         q=packed["slots"] if MODE == "grp" else packed["q"],
                         NSEG=packed["NSEG"], SEG=packed["SEG"],
                         QCAP=GRP_QCAP if MODE == "grp" else 8,
                         scratch=GRP_SCRATCH if MODE == "grp" else 16384)


def kernel(x, edge_index, batch, W, b, prelu_a):
    x = np.asarray(x)
    edge_index = np.asarray(edge_index)
    batch = np.asarray(batch)
    packed = pack_inputs(x, edge_index, batch)
    nc = build_from_packed(packed)
    in_maps = make_in_maps(packed, np.asarray(W), np.asarray(b),
                           np.asarray(prelu_a), mode=MODE)
    res = run_bass_kernel_spmd(nc, in_maps, core_ids=list(range(NCORES)))
    return np.asarray(res.results[0]["pooled"], np.float32)

